# revision 51
# baseline (speedup 1.0000x reference)
"""GQA attention (B=2,S=2048,D=1024,H=16,KH=4,HD=64) + RoPE + causal mask on 8 trn2 cores.

Sharding: core = (batch b, kv-group g).  Each core computes its 4 query heads'
attention against its single KV head and a partial output  O_g @ wo_g  [S, D];
the host sums the 4 partials per batch.

Per-core device pipeline (everything transposed so softmax-sum runs on the PE):
  - host passes x[b]^T so QKV projections contract D on partitions
  - head_dim of wq/wk is permuted on host to [evens, odds] so RoPE is two
    32-row blocks; scores are invariant to a consistent q/k head_dim permutation
  - RoPE: DVE evac (f32 PSUM -> bf16 SBUF), 4 partition-swap DMAs, two bf16
    DVE muls with [cos..]/[-sin,+sin..] tiles, add on GpSimd
  - scores computed transposed  S^T[k, q] = K^T(lhsT) x Q^T(rhs), bf16 matmuls
  - causal handling: per 128-key tile only the q >= k_tile_start slice is
    computed/exp'd; the one triangular 128x128 block per tile is zeroed
    AFTER exp by a cheap DVE mul with a 0/1 triangle tile (no mask matmuls)
  - softmax without max-subtraction (scores bounded); exp on ScalarE w/ scale=1/8
  - AV uses V augmented with a ones column: one accumulating matmul per key
    tile yields both O^T[64, q] and the softmax denominator row
  - normalization: DVE reciprocal of denom row, partition-broadcast DMA of
    1/denom, fused multiply on the PSUM->SBUF copy; even heads' multiply
    writes the packed OTC tile rows 0..64 directly (odd heads need one
    partition-move DMA to rows 64..128 - matmul out base must be 0/32/64)
  - wo projection consumes O^T chunks directly as lhsT; PSUM -> SBUF pair ->
    one DRAM store per 128-row block; emission interleaves wo blocks between
    late attention chunks so the PE stays fed through the tail
"""

import os
import sys

import numpy as np

for _p in ("/opt/trn_rl_repo", "/root/.axon_site/_ro/trn_rl_repo"):
    if os.path.isdir(_p) and _p not in sys.path:
        sys.path.insert(0, _p)

from contextlib import ExitStack

import concourse.bass as bass
import concourse.tile as tile
from concourse import bacc as _bacc
from concourse import mybir
from concourse.bass_utils import run_bass_kernel_spmd

B, S, D = 2, 2048, 1024
H, KH, HD = 16, 4, 64
REP = H // KH          # 4 query heads per kv head
GH = REP               # heads per core
P = 128
QB = 512               # q block (matmul moving free dim)
NKT = S // P           # 16 key tiles
NQB = S // QB          # 4 q blocks
DCH = D // P           # 8 contraction chunks for D

f32 = mybir.dt.float32
f32r = mybir.dt.float32r
bf16 = mybir.dt.bfloat16

LAST_EXEC_NS = None
LAST_PROFILE = None


def _classify_mask(mask):
    m = np.asarray(mask).reshape(S, S)
    if not m.any():
        return "none"
    tril = np.tril(np.ones((S, S), dtype=bool))
    if (m[tril] == 0.0).all() and (m[~tril] < -1e30).all():
        return "causal"
    return "general"


def _build_nc(mode):
    import os as _os
    _skip = set(_os.environ.get("KABLATE", "").split(","))
    nc = bass.Bass()
    xT = nc.declare_dram_parameter("xT", [DCH, P, S], bf16, isOutput=False)
    wq = nc.declare_dram_parameter("wq", [DCH, P, GH * HD], bf16, isOutput=False)
    wk = nc.declare_dram_parameter("wk", [DCH, P, 2 * HD], bf16, isOutput=False)
    wv = nc.declare_dram_parameter("wv", [DCH, P, HD], bf16, isOutput=False)
    wo = nc.declare_dram_parameter("wo", [2, P, D], bf16, isOutput=False)
    cos = nc.declare_dram_parameter("cos", [P, S], bf16, isOutput=False)
    sin = nc.declare_dram_parameter("sin", [P, S], bf16, isOutput=False)
    tri = nc.declare_dram_parameter("tri", [P, P], bf16, isOutput=False)
    if mode == "general":
        maskT = nc.declare_dram_parameter("maskT", [NKT, P, S], f32, isOutput=False)
    out = nc.declare_dram_parameter("out", [S, D], f32, isOutput=True)

    with tile.TileContext(nc) as tc, ExitStack() as ctx:
        const = ctx.enter_context(tc.tile_pool(name="const", bufs=1))
        big = ctx.enter_context(tc.tile_pool(name="big", bufs=1))
        work = ctx.enter_context(tc.tile_pool(name="work", bufs=6))
        ptp = ctx.enter_context(tc.tile_pool(name="ptp", bufs=8))
        psp = ctx.enter_context(tc.tile_pool(name="psp", bufs=2, space="PSUM"))
        stp = ctx.enter_context(tc.tile_pool(name="stp", bufs=2, space="PSUM"))
        avp = ctx.enter_context(tc.tile_pool(name="avp", bufs=2, space="PSUM"))

        # ---- constants / weights to SBUF ----
        # few DMAs per tile: consumers wait per DMA-queue semaphore, and walrus
        # rejects instructions with too many wait conditions
        # preload order matters: the first Q projection + rope + first scores
        # need wq/xt[sb0]/wk/wv/tri/cos/sin - queue those first so the PE
        # doesn't idle behind the bulk of the xt/wo loads
        xt_sb = big.tile([P, DCH, S], bf16, tag="xt")
        wq_sb = const.tile([P, DCH, GH * HD], bf16, tag="wq")
        wk_sb = const.tile([P, DCH, 2 * HD], bf16, tag="wk")
        wv_sb = const.tile([P, DCH, HD], bf16, tag="wv")
        wo_sb = const.tile([P, 2, D], bf16, tag="wo")
        cos_sb = const.tile([P, S], bf16, tag="cos")
        sin_sb = const.tile([P, S], bf16, tag="sin")
        tri_sb = const.tile([P, P], bf16, tag="tri")

        def _xt_load(sb, half=None):
            sl = slice(sb * QB, (sb + 1) * QB)
            ch = slice(0, DCH) if half is None else (
                slice(0, DCH // 2) if half == 0 else slice(DCH // 2, DCH))
            nc.sync.dma_start(
                out=xt_sb[:, ch, sl],
                in_=xT[ch, :, sl].rearrange("c p s -> p c s"),
            )
        nc.sync.dma_start(out=wq_sb, in_=wq[:, :, :].rearrange("c p f -> p c f"))
        _xt_load(0, half=0)
        _xt_load(0, half=1)
        nc.sync.dma_start(out=wk_sb, in_=wk[:, :, :].rearrange("c p f -> p c f"))
        nc.sync.dma_start(out=cos_sb, in_=cos[:, :])
        nc.sync.dma_start(out=wv_sb, in_=wv[:, :, :].rearrange("c p f -> p c f"))
        nc.sync.dma_start(out=sin_sb, in_=sin[:, :])
        nc.sync.dma_start(out=tri_sb, in_=tri[:, :])
        for _sb in range(1, NQB):
            _xt_load(_sb)
        nc.sync.dma_start(out=wo_sb, in_=wo[:, :, :].rearrange("c p f -> p c f"))

        # per-s-block tiles: fine-grained deps let attention start as soon as
        # the first s-block of Q/K/V is ready instead of after all of stage A
        QT_t = [big.tile([P, 2, QB], bf16, tag=f"QT{i}", name=f"QT{i}") for i in range(NQB)]
        KT_t = [big.tile([P, QB], bf16, tag=f"KT{i}", name=f"KT{i}") for i in range(NQB)]
        # V augmented with a ones column: cols [V(64), ones]
        V_t = [big.tile([P, 4, HD + 1], bf16, tag=f"V{i}", name=f"V{i}") for i in range(NQB)]
        OTC_t = [big.tile([P, 2, QB], bf16, tag=f"OTC{i}", name=f"OTC{i}") for i in range(NQB)]
        for i in range(NQB):
            nc.vector.memset(V_t[i][:, :, HD:HD + 1], 1.0)

        def rope(ps, out_ap, sl):
            # ps rows: per 64-group [evens(32), odds(32)]; swap 32-row halves.
            # DMA cannot read PSUM, so evacuate via DVE copy (cast to bf16).
            sb_ps = work.tile([P, QB], bf16, tag="ropesb")
            nc.vector.tensor_copy(sb_ps, ps)
            tmp = work.tile([P, QB], bf16, tag="ropetmp")
            for r0 in range(0, P, 64):
                nc.sync.dma_start(
                    out=tmp[r0:r0 + 32, :], in_=sb_ps[r0 + 32:r0 + 64, :])
                nc.sync.dma_start(
                    out=tmp[r0 + 32:r0 + 64, :], in_=sb_ps[r0:r0 + 32, :])
            ta = work.tile([P, QB], bf16, tag="ropeta")
            tb = work.tile([P, QB], bf16, tag="ropetb")
            nc.vector.tensor_mul(ta, sb_ps, cos_sb[:, sl])
            nc.vector.tensor_mul(tb, tmp, sin_sb[:, sl])
            nc.gpsimd.tensor_add(out_ap, ta, tb)

        # ---- Q/K/V per s-block (emission order lets qb0 attention start early)
        def emit_stage_a_sb(sb):
          if True:
              sl = slice(sb * QB, (sb + 1) * QB)
              for ch in range(2):
                  ps = psp.tile([P, QB], f32, tag="proj")
                  for dc in range(DCH):
                      nc.tensor.matmul(
                          ps, lhsT=wq_sb[:, dc, ch * P:(ch + 1) * P],
                          rhs=xt_sb[:, dc, sl],
                          start=(dc == 0), stop=(dc == DCH - 1),
                      )
                  rope(ps, QT_t[sb][:, ch, :], sl)
              ps = psp.tile([P, QB], f32, tag="proj")
              for dc in range(DCH):
                  nc.tensor.matmul(
                      ps, lhsT=wk_sb[:, dc, :], rhs=xt_sb[:, dc, sl],
                      start=(dc == 0), stop=(dc == DCH - 1),
                  )
              rope(ps, KT_t[sb], sl)
              for st_i in range(4 * sb, 4 * sb + 4):
                  ps = psp.tile([P, HD], f32, tag="proj")
                  for dc in range(DCH):
                      nc.tensor.matmul(
                          ps, lhsT=xt_sb[:, dc, st_i * P:(st_i + 1) * P],
                          rhs=wv_sb[:, dc, :],
                          start=(dc == 0), stop=(dc == DCH - 1),
                      )
                  nc.vector.tensor_copy(V_t[sb][:, st_i - 4 * sb, 0:HD], ps)

        # ---- attention per (head, q block), emitted in wavefront order ----
        def emit_attn(qb, heads=None, tail=False):
            # odd head first within each chunk: the chunk's last OTC write is
            # then the even head's direct DVE write (no partition-move DMA)
            for h in (heads if heads is not None
                      else ([1, 0, 3, 2] if "B" not in _skip else [])):
                ch, hr = h // 2, (h % 2) * 64
                q0 = qb * QB
                qsl = slice(q0, q0 + QB)
                nk = 4 * (qb + 1) if mode == "causal" else NKT
                # AV rows: [O(0..64), denom@64] <- V cols [V, ones]; for even
                # heads (hr=0) the normalize mul then writes OTC rows 0..64
                # directly; odd heads need a partition-move DMA to rows 64..128
                # (matmul out base partition must be 0/32/64, so the odd-head
                # result cannot be placed at its OTC rows directly)
                av = avp.tile([P, QB], f32, tag="av")
                av_out = av[0:HD + 1, :]
                denom, orows = av[HD:HD + 1, :], av[0:HD, :]
                for kt0 in range(0, nk, 2):
                    st = stp.tile([P, 2, QB], f32, tag="st")
                    offs = []
                    for j in range(2):
                        kt = kt0 + j
                        # causal: keys [128kt, 128kt+128) only see q >= 128kt
                        # (within this q block) -> slice the moving dim
                        diag = mode == "causal" and kt >= nk - 4
                        o = (kt - (nk - 4)) * P if diag else 0
                        offs.append(o)
                        nc.tensor.matmul(
                            st[:, j, o:],
                            lhsT=KT_t[kt // 4][hr:hr + 64,
                                               (kt % 4) * P:(kt % 4 + 1) * P],
                            rhs=QT_t[qb][hr:hr + 64, ch, o:],
                            start=True, stop=True,
                        )
                        if mode == "general":
                            mt = work.tile([P, QB], f32, tag="maskt")
                            nc.sync.dma_start(out=mt, in_=maskT[kt, :, qsl])
                            nc.vector.tensor_add(st[:, j, :], st[:, j, :], mt)
                    if "E" in _skip:
                        continue
                    pt = ptp.tile([P, 2, QB], bf16, tag="pt")
                    if offs[0] == offs[1]:
                        nc.scalar.activation(
                            pt[:, :, offs[0]:], st[:, :, offs[0]:],
                            mybir.ActivationFunctionType.Exp, scale=0.125,
                        )
                    else:
                        for j in range(2):
                            nc.scalar.activation(
                                pt[:, j, offs[j]:], st[:, j, offs[j]:],
                                mybir.ActivationFunctionType.Exp, scale=0.125,
                            )
                    for j in range(2):
                        kt = kt0 + j
                        o = offs[j]
                        if mode == "causal" and kt >= nk - 4:
                            # zero the above-diagonal part of the one
                            # triangular 128x128 block (post-exp mask)
                            nc.vector.tensor_mul(
                                pt[:, j, o:o + P], pt[:, j, o:o + P], tri_sb)
                        nc.tensor.matmul(
                            av_out[:, o:],
                            lhsT=V_t[kt // 4][:, kt % 4, :],
                            rhs=pt[:, j, o:],
                            start=(kt == 0), stop=(kt == nk - 1),
                        )
                # normalize: r = 1/denom; replicate across 64 partitions with a
                # partition-step-0 DMA source AP; the fused multiply on the
                # PSUM->SBUF copy writes the packed OTC tile directly
                r1 = work.tile([1, QB], f32, tag="r1")
                rbs = work.tile([64, QB], f32, tag="rbs")
                ot = (None if hr == 0 else
                      work.tile([64, QB], bf16, tag="ot"))
                # tail heads split the normalize into column halves: denom
                # cols < 128*(o+1) are final before the last AV matmuls, so
                # the first half's recip->bcast->mul chain hides under them
                # and half the wo stop-matmuls unblock earlier
                for hsl in ([slice(0, QB // 2), slice(QB // 2, QB)]
                            if tail else [slice(0, QB)]):
                    nc.vector.reciprocal(r1[:, hsl], denom[:, hsl])
                    # replicate 1/denom across 64 partitions with a
                    # partition-step-0 DMA source AP (SWDGE: own queue, keeps
                    # the SP/HWDGE stream free; a DVE op may read only one
                    # non-scalar PSUM operand -> broadcast must land in SBUF)
                    r1h = r1[:, hsl]
                    r1b = bass.AP(tensor=r1h.tensor, offset=r1h.offset,
                                  ap=[list(r1h.ap[0]), [0, 64]]
                                  + [list(a) for a in r1h.ap[1:]])
                    with nc.allow_non_contiguous_dma(
                            reason="partition broadcast"):
                        # tail chains use HWDGE (lower latency; SP idle there)
                        (nc.sync if tail else nc.gpsimd).dma_start(
                            out=rbs[:, hsl], in_=r1b)
                    if hr == 0:
                        nc.vector.tensor_mul(
                            OTC_t[qb][0:64, ch, hsl], orows[:, hsl],
                            rbs[:, hsl])
                    else:
                        nc.vector.tensor_mul(ot[:, hsl], orows[:, hsl],
                                             rbs[:, hsl])
                if hr != 0:
                    # gpsimd SWDGE is pinned to one queue -> single wait
                    # condition for the wo matmuls that consume OTC
                    nc.gpsimd.dma_start(out=OTC_t[qb][64:128, ch, :], in_=ot)

        # ---- output projection: out[q,:] = sum_c OTC[:,c,q].T @ wo[c] ----
        def emit_wo(qb, corder=(0, 1), alt_evac=False):
            if "C" in _skip:
                return
            for qt in range(4 * qb, 4 * qb + 4):
                osb = work.tile([P, 2, 512], f32, tag="osb")
                for dh in range(2):
                    ps = psp.tile([P, QB], f32, tag="proj")
                    for ci, c in enumerate(corder):
                        nc.tensor.matmul(
                            ps[:, 0:512],
                            lhsT=OTC_t[qt // 4][:, c,
                                                (qt % 4) * P:(qt % 4 + 1) * P],
                            rhs=wo_sb[:, c, dh * 512:(dh + 1) * 512],
                            start=(ci == 0), stop=(ci == 1),
                        )
                    # at the kernel tail ACT is idle: alternate evacuations
                    # across DVE/ACT so the last copies run in parallel
                    if alt_evac and dh == 1:
                        nc.scalar.copy(osb[:, dh, :], ps[:, 0:512])
                    else:
                        nc.vector.tensor_copy(osb[:, dh, :], ps[:, 0:512])
                nc.sync.dma_start(
                    out=out[qt * P:(qt + 1) * P, :], in_=osb)

        # offset-by-one interleave: attention for qb emitted after stage-A
        # block qb+1, so projections keep a one-block head start on the PE
        if mode == "causal":
            # attn(qb) only reads KT/V s-blocks <= qb, all emitted beforehand;
            # wo(qb) interleaved as soon as OTC[qb] is complete so the PE has
            # independent work while late attention chains drain
            emit_stage_a_sb(0)
            emit_attn(0, heads=[1, 0])
            emit_stage_a_sb(1)
            emit_attn(0, heads=[3, 2])
            emit_attn(1, heads=[1, 0])
            emit_stage_a_sb(2)
            emit_attn(1, heads=[3, 2])
            emit_attn(2, heads=[1, 0])
            emit_stage_a_sb(3)
            emit_attn(2, heads=[3, 2])
            emit_wo(0)
            emit_attn(3, heads=[3, 2])
            emit_wo(1)
            emit_attn(3, heads=[1, 0], tail=True)
            emit_wo(2)
            emit_wo(3, corder=(1, 0), alt_evac=True)
        else:
            # non-causal attn reads ALL KT/V tiles: emitting it early would
            # precede their writers (Tile records deps at emission time)
            for _sb in range(NQB):
                emit_stage_a_sb(_sb)
            for _qb in range(NQB):
                emit_attn(_qb)
            for _qb in range(NQB):
                emit_wo(_qb)
    # split multi-wait conditions: TRN2 instructions hold at most one sync
    # wait (EventSemaphore holds two); walrus refuses to split them itself
    import bass_rust
    bass_rust.move_matmul_waits_to_ldweights(nc.m)
    bass_rust.generate_event_semaphores(nc)
    return nc


_NC_CACHE = {}


def kernel(_trace=False, **inputs):
    global LAST_EXEC_NS, LAST_PROFILE
    x = np.ascontiguousarray(np.asarray(inputs["x"], dtype=np.float32))
    wq = np.asarray(inputs["wq"], dtype=np.float32)
    wk = np.asarray(inputs["wk"], dtype=np.float32)
    wv = np.asarray(inputs["wv"], dtype=np.float32)
    wo = np.asarray(inputs["wo"], dtype=np.float32)
    fc = np.asarray(inputs["freqs_cos"], dtype=np.float32)
    fs = np.asarray(inputs["freqs_sin"], dtype=np.float32)
    mask = np.asarray(inputs["mask"], dtype=np.float32)

    mode = _classify_mask(mask)
    if mode not in _NC_CACHE:
        _NC_CACHE[mode] = _build_nc(mode)
    nc = _NC_CACHE[mode]
    in_maps = _make_in_maps(x, wq, wk, wv, wo, fc, fs, mask, mode)

    try:
        res = run_bass_kernel_spmd(
            nc, in_maps, core_ids=list(range(8)), trace=_trace)
    except (ModuleNotFoundError, ImportError):
        res = run_bass_kernel_spmd(
            nc, in_maps, core_ids=list(range(8)), trace=False)
    LAST_EXEC_NS = res.exec_time_ns
    LAST_PROFILE = res.profile_json
    full = np.zeros((B, S, D), dtype=np.float32)
    for b in range(B):
        for g in range(KH):
            full[b] += res.results[b * KH + g]["out"]
    return full


def _make_in_maps(x, wq, wk, wv, wo, fc, fs, mask, mode):
    # head_dim permutation: evens then odds (consistent on q & k -> scores invariant)
    perm = np.concatenate([np.arange(0, HD, 2), np.arange(1, HD, 2)])
    wq_p = wq.reshape(D, H, HD)[:, :, perm].reshape(D, H * HD)
    wk_p = wk.reshape(D, KH, HD)[:, :, perm].reshape(D, KH * HD)

    cosT = fc.T.astype(np.float32)                      # [32, S]
    sinT = fs.T.astype(np.float32)
    cos_rep = np.ascontiguousarray(np.tile(cosT, (4, 1)))          # [128, S]
    sin_signed = np.ascontiguousarray(
        np.concatenate([-sinT, sinT, -sinT, sinT], axis=0))        # [128, S]

    # tri[c, i] = 1 where q-offset i >= key-offset c (keep), else 0
    cc = np.arange(P)[:, None]
    ii = np.arange(P)[None, :]
    tri = (ii >= cc).astype(np.float32)

    import ml_dtypes
    b16 = ml_dtypes.bfloat16

    in_maps = []
    for b in range(B):
        xTb = np.ascontiguousarray(x[b].T).astype(b16).reshape(DCH, P, S)
        for g in range(KH):
            wk_g = wk_p[:, g * HD:(g + 1) * HD]
            wk_dup = np.concatenate([wk_g, wk_g], axis=1)       # [D, 128]
            m = {
                "xT": xTb,
                "wq": np.ascontiguousarray(
                    wq_p[:, g * GH * HD:(g + 1) * GH * HD]
                ).astype(b16).reshape(DCH, P, GH * HD),
                "wk": np.ascontiguousarray(wk_dup).astype(b16).reshape(DCH, P, 2 * HD),
                "wv": np.ascontiguousarray(
                    wv[:, g * HD:(g + 1) * HD]).astype(b16).reshape(DCH, P, HD),
                "wo": np.ascontiguousarray(
                    wo[g * GH * HD:(g + 1) * GH * HD]).astype(b16).reshape(2, P, D),
                "cos": cos_rep.astype(b16),
                "sin": sin_signed.astype(b16),
                "tri": tri.astype(b16),
            }
            if mode == "general":
                m["maskT"] = np.ascontiguousarray(
                    mask.reshape(S, S).T).reshape(NKT, P, S)
            in_maps.append(m)
    return in_maps


# revision 57
# speedup vs baseline: 1.0456x; 1.0456x over previous
"""GQA attention (B=2,S=2048,D=1024,H=16,KH=4,HD=64) + RoPE + causal mask on 8 trn2 cores.

Sharding: core = (batch b, kv-group g).  Each core computes its 4 query heads'
attention against its single KV head and a partial output  O_g @ wo_g  [S, D];
the host sums the 4 partials per batch.

Per-core device pipeline (everything transposed so softmax-sum runs on the PE):
  - host passes x[b]^T so QKV projections contract D on partitions
  - head_dim of wq/wk is permuted on host to [evens, odds] so RoPE is two
    32-row blocks; scores are invariant to a consistent q/k head_dim permutation
  - RoPE: DVE evac (f32 PSUM -> bf16 SBUF), 4 partition-swap DMAs, two bf16
    DVE muls with [cos..]/[-sin,+sin..] tiles, add on GpSimd
  - scores computed transposed  S^T[k, q] = K^T(lhsT) x Q^T(rhs), bf16 matmuls
  - causal handling: per 128-key tile only the q >= k_tile_start slice is
    computed/exp'd; the one triangular 128x128 block per tile is zeroed
    AFTER exp by a cheap DVE mul with a 0/1 triangle tile (no mask matmuls)
  - softmax without max-subtraction (scores bounded); exp on ScalarE w/ scale=1/8
  - AV uses V augmented with a ones column: one accumulating matmul per key
    tile yields both O^T[64, q] and the softmax denominator row
  - normalization: DVE reciprocal of denom row, partition-broadcast DMA of
    1/denom, fused multiply on the PSUM->SBUF copy; even heads' multiply
    writes the packed OTC tile rows 0..64 directly (odd heads need one
    partition-move DMA to rows 64..128 - matmul out base must be 0/32/64)
  - wo projection consumes O^T chunks directly as lhsT; PSUM -> SBUF pair ->
    one DRAM store per 128-row block; emission interleaves wo blocks between
    late attention chunks so the PE stays fed through the tail
"""

import os
import sys

import numpy as np

for _p in ("/opt/trn_rl_repo", "/root/.axon_site/_ro/trn_rl_repo"):
    if os.path.isdir(_p) and _p not in sys.path:
        sys.path.insert(0, _p)

from contextlib import ExitStack

import concourse.bass as bass
import concourse.tile as tile
from concourse import bacc as _bacc
from concourse import mybir
from concourse.bass_utils import run_bass_kernel_spmd

B, S, D = 2, 2048, 1024
H, KH, HD = 16, 4, 64
REP = H // KH          # 4 query heads per kv head
GH = REP               # heads per core
P = 128
QB = 512               # q block (matmul moving free dim)
NKT = S // P           # 16 key tiles
NQB = S // QB          # 4 q blocks
DCH = D // P           # 8 contraction chunks for D

f32 = mybir.dt.float32
f32r = mybir.dt.float32r
bf16 = mybir.dt.bfloat16

LAST_EXEC_NS = None
LAST_PROFILE = None


def _classify_mask(mask):
    m = np.asarray(mask).reshape(S, S)
    if not m.any():
        return "none"
    tril = np.tril(np.ones((S, S), dtype=bool))
    if (m[tril] == 0.0).all() and (m[~tril] < -1e30).all():
        return "causal"
    return "general"


def _build_nc(mode):
    import os as _os
    _skip = set(_os.environ.get("KABLATE", "").split(","))
    nc = bass.Bass()
    xT = nc.declare_dram_parameter("xT", [DCH, P, S], bf16, isOutput=False)
    wq = nc.declare_dram_parameter("wq", [DCH, P, GH * HD], bf16, isOutput=False)
    wk = nc.declare_dram_parameter("wk", [DCH, P, 2 * HD], bf16, isOutput=False)
    wv = nc.declare_dram_parameter("wv", [DCH, P, HD], bf16, isOutput=False)
    wo = nc.declare_dram_parameter("wo", [2, P, D], bf16, isOutput=False)
    cos = nc.declare_dram_parameter("cos", [P, S], bf16, isOutput=False)
    sin = nc.declare_dram_parameter("sin", [P, S], bf16, isOutput=False)
    tri = nc.declare_dram_parameter("tri", [P, P], bf16, isOutput=False)
    if mode == "general":
        maskT = nc.declare_dram_parameter("maskT", [NKT, P, S], f32, isOutput=False)
    out = nc.declare_dram_parameter("out", [S, D], f32, isOutput=True)

    with tile.TileContext(nc) as tc, ExitStack() as ctx:
        const = ctx.enter_context(tc.tile_pool(name="const", bufs=1))
        big = ctx.enter_context(tc.tile_pool(name="big", bufs=1))
        work = ctx.enter_context(tc.tile_pool(name="work", bufs=6))
        ptp = ctx.enter_context(tc.tile_pool(name="ptp", bufs=8))
        psp = ctx.enter_context(tc.tile_pool(name="psp", bufs=2, space="PSUM"))
        stp = ctx.enter_context(tc.tile_pool(name="stp", bufs=2, space="PSUM"))
        avp = ctx.enter_context(tc.tile_pool(name="avp", bufs=2, space="PSUM"))

        # ---- constants / weights to SBUF ----
        # few DMAs per tile: consumers wait per DMA-queue semaphore, and walrus
        # rejects instructions with too many wait conditions
        # preload order matters: the first Q projection + rope + first scores
        # need wq/xt[sb0]/wk/wv/tri/cos/sin - queue those first so the PE
        # doesn't idle behind the bulk of the xt/wo loads
        xt_sb = big.tile([P, DCH, S], bf16, tag="xt")
        wq_sb = const.tile([P, DCH, GH * HD], bf16, tag="wq")
        wk_sb = const.tile([P, DCH, 2 * HD], bf16, tag="wk")
        wv_sb = const.tile([P, DCH, HD], bf16, tag="wv")
        wo_sb = const.tile([P, 2, D], bf16, tag="wo")
        cos_sb = const.tile([P, S], bf16, tag="cos")
        sin_sb = const.tile([P, S], bf16, tag="sin")
        tri_sb = const.tile([P, P], bf16, tag="tri")

        def _xt_load(sb, half=None):
            sl = slice(sb * QB, (sb + 1) * QB)
            ch = slice(0, DCH) if half is None else (
                slice(0, DCH // 2) if half == 0 else slice(DCH // 2, DCH))
            nc.sync.dma_start(
                out=xt_sb[:, ch, sl],
                in_=xT[ch, :, sl].rearrange("c p s -> p c s"),
            )
        nc.sync.dma_start(out=wq_sb, in_=wq[:, :, :].rearrange("c p f -> p c f"))
        _xt_load(0, half=0)
        _xt_load(0, half=1)
        nc.sync.dma_start(out=wk_sb, in_=wk[:, :, :].rearrange("c p f -> p c f"))
        nc.sync.dma_start(out=cos_sb, in_=cos[:, :])
        nc.sync.dma_start(out=wv_sb, in_=wv[:, :, :].rearrange("c p f -> p c f"))
        nc.sync.dma_start(out=sin_sb, in_=sin[:, :])
        nc.sync.dma_start(out=tri_sb, in_=tri[:, :])
        for _sb in range(1, NQB):
            _xt_load(_sb)
        nc.sync.dma_start(out=wo_sb, in_=wo[:, :, :].rearrange("c p f -> p c f"))

        # per-s-block tiles: fine-grained deps let attention start as soon as
        # the first s-block of Q/K/V is ready instead of after all of stage A
        QT_t = [big.tile([P, 2, QB], bf16, tag=f"QT{i}", name=f"QT{i}") for i in range(NQB)]
        KT_t = [big.tile([P, QB], bf16, tag=f"KT{i}", name=f"KT{i}") for i in range(NQB)]
        # V augmented with a ones column: cols [V(64), ones]
        V_t = [big.tile([P, 4, HD + 1], bf16, tag=f"V{i}", name=f"V{i}") for i in range(NQB)]
        OTC_t = [big.tile([P, 2, QB], bf16, tag=f"OTC{i}", name=f"OTC{i}") for i in range(NQB)]
        for i in range(NQB):
            nc.vector.memset(V_t[i][:, :, HD:HD + 1], 1.0)

        def rope(ps, out_ap, sl):
            # ps rows: per 64-group [evens(32), odds(32)]; swap 32-row halves.
            # DMA cannot read PSUM, so evacuate via DVE copy (cast to bf16).
            sb_ps = work.tile([P, QB], bf16, tag="ropesb")
            nc.vector.tensor_copy(sb_ps, ps)
            tmp = work.tile([P, QB], bf16, tag="ropetmp")
            for r0 in range(0, P, 64):
                nc.sync.dma_start(
                    out=tmp[r0:r0 + 32, :], in_=sb_ps[r0 + 32:r0 + 64, :])
                nc.sync.dma_start(
                    out=tmp[r0 + 32:r0 + 64, :], in_=sb_ps[r0:r0 + 32, :])
            ta = work.tile([P, QB], bf16, tag="ropeta")
            tb = work.tile([P, QB], bf16, tag="ropetb")
            nc.vector.tensor_mul(ta, sb_ps, cos_sb[:, sl])
            nc.vector.tensor_mul(tb, tmp, sin_sb[:, sl])
            nc.gpsimd.tensor_add(out_ap, ta, tb)

        # ---- Q/K/V per s-block (emission order lets qb0 attention start early)
        def emit_stage_a_sb(sb):
          if True:
              sl = slice(sb * QB, (sb + 1) * QB)
              for ch in range(2):
                  ps = psp.tile([P, QB], f32, tag="proj")
                  for dc in range(DCH):
                      nc.tensor.matmul(
                          ps, lhsT=wq_sb[:, dc, ch * P:(ch + 1) * P],
                          rhs=xt_sb[:, dc, sl],
                          start=(dc == 0), stop=(dc == DCH - 1),
                      )
                  rope(ps, QT_t[sb][:, ch, :], sl)
              ps = psp.tile([P, QB], f32, tag="proj")
              for dc in range(DCH):
                  nc.tensor.matmul(
                      ps, lhsT=wk_sb[:, dc, :], rhs=xt_sb[:, dc, sl],
                      start=(dc == 0), stop=(dc == DCH - 1),
                  )
              rope(ps, KT_t[sb], sl)
              for st_i in range(4 * sb, 4 * sb + 4):
                  ps = psp.tile([P, HD], f32, tag="proj")
                  for dc in range(DCH):
                      nc.tensor.matmul(
                          ps, lhsT=xt_sb[:, dc, st_i * P:(st_i + 1) * P],
                          rhs=wv_sb[:, dc, :],
                          start=(dc == 0), stop=(dc == DCH - 1),
                      )
                  nc.vector.tensor_copy(V_t[sb][:, st_i - 4 * sb, 0:HD], ps)

        # ---- attention per (head, q block), emitted in wavefront order ----
        def emit_attn(qb, heads=None, tail=False):
            # odd head first within each chunk: the chunk's last OTC write is
            # then the even head's direct DVE write (no partition-move DMA)
            for h in (heads if heads is not None
                      else ([1, 0, 3, 2] if "B" not in _skip else [])):
                ch, hr = h // 2, (h % 2) * 64
                q0 = qb * QB
                qsl = slice(q0, q0 + QB)
                nk = 4 * (qb + 1) if mode == "causal" else NKT
                # AV rows: [O(0..64), denom@64] <- V cols [V, ones]; for even
                # heads (hr=0) the normalize mul then writes OTC rows 0..64
                # directly; odd heads need a partition-move DMA to rows 64..128
                # (matmul out base partition must be 0/32/64, so the odd-head
                # result cannot be placed at its OTC rows directly)
                av = avp.tile([P, QB], f32, tag="av")
                av_out = av[0:HD + 1, :]
                denom, orows = av[HD:HD + 1, :], av[0:HD, :]
                for kt0 in range(0, nk, 2):
                    st = stp.tile([P, 2, QB], f32, tag="st")
                    offs = []
                    for j in range(2):
                        kt = kt0 + j
                        # causal: keys [128kt, 128kt+128) only see q >= 128kt
                        # (within this q block) -> slice the moving dim
                        diag = mode == "causal" and kt >= nk - 4
                        o = (kt - (nk - 4)) * P if diag else 0
                        offs.append(o)
                        nc.tensor.matmul(
                            st[:, j, o:],
                            lhsT=KT_t[kt // 4][hr:hr + 64,
                                               (kt % 4) * P:(kt % 4 + 1) * P],
                            rhs=QT_t[qb][hr:hr + 64, ch, o:],
                            start=True, stop=True,
                        )
                        if mode == "general":
                            mt = work.tile([P, QB], f32, tag="maskt")
                            nc.sync.dma_start(out=mt, in_=maskT[kt, :, qsl])
                            nc.vector.tensor_add(st[:, j, :], st[:, j, :], mt)
                    if "E" in _skip:
                        continue
                    pt = ptp.tile([P, 2, QB], bf16, tag="pt")
                    if offs[0] == offs[1]:
                        nc.scalar.activation(
                            pt[:, :, offs[0]:], st[:, :, offs[0]:],
                            mybir.ActivationFunctionType.Exp, scale=0.125,
                        )
                    else:
                        for j in range(2):
                            nc.scalar.activation(
                                pt[:, j, offs[j]:], st[:, j, offs[j]:],
                                mybir.ActivationFunctionType.Exp, scale=0.125,
                            )
                    for j in range(2):
                        kt = kt0 + j
                        o = offs[j]
                        if mode == "causal" and kt >= nk - 4:
                            # zero the above-diagonal part of the one
                            # triangular 128x128 block (post-exp mask)
                            nc.vector.tensor_mul(
                                pt[:, j, o:o + P], pt[:, j, o:o + P], tri_sb)
                        nc.tensor.matmul(
                            av_out[:, o:],
                            lhsT=V_t[kt // 4][:, kt % 4, :],
                            rhs=pt[:, j, o:],
                            start=(kt == 0), stop=(kt == nk - 1),
                        )
                # normalize: r = 1/denom; replicate across 64 partitions with a
                # partition-step-0 DMA source AP; the fused multiply on the
                # PSUM->SBUF copy writes the packed OTC tile directly
                r1 = work.tile([1, QB], f32, tag="r1")
                rbs = work.tile([64, QB], f32, tag="rbs")
                ot = (None if hr == 0 else
                      work.tile([64, QB], bf16, tag="ot"))
                # tail heads split the normalize into column halves: denom
                # cols < 128*(o+1) are final before the last AV matmuls, so
                # the first half's recip->bcast->mul chain hides under them
                # and half the wo stop-matmuls unblock earlier
                for hsl in ([slice(0, QB // 2), slice(QB // 2, QB)]
                            if tail else [slice(0, QB)]):
                    nc.vector.reciprocal(r1[:, hsl], denom[:, hsl])
                    # replicate 1/denom across 64 partitions with a
                    # partition-step-0 DMA source AP (SWDGE: own queue, keeps
                    # the SP/HWDGE stream free; a DVE op may read only one
                    # non-scalar PSUM operand -> broadcast must land in SBUF)
                    r1h = r1[:, hsl]
                    r1b = bass.AP(tensor=r1h.tensor, offset=r1h.offset,
                                  ap=[list(r1h.ap[0]), [0, 64]]
                                  + [list(a) for a in r1h.ap[1:]])
                    with nc.allow_non_contiguous_dma(
                            reason="partition broadcast"):
                        # tail chains use HWDGE (lower latency; SP idle there)
                        (nc.sync if tail else nc.gpsimd).dma_start(
                            out=rbs[:, hsl], in_=r1b)
                    if hr == 0:
                        nc.vector.tensor_mul(
                            OTC_t[qb][0:64, ch, hsl], orows[:, hsl],
                            rbs[:, hsl])
                    else:
                        nc.vector.tensor_mul(ot[:, hsl], orows[:, hsl],
                                             rbs[:, hsl])
                if hr != 0:
                    # gpsimd SWDGE is pinned to one queue -> single wait
                    # condition for the wo matmuls that consume OTC
                    nc.gpsimd.dma_start(out=OTC_t[qb][64:128, ch, :], in_=ot)

        # ---- output projection: out[q,:] = sum_c OTC[:,c,q].T @ wo[c] ----
        def emit_wo(qb, corder=(0, 1), alt_evac=False):
            if "C" in _skip:
                return
            for qt in range(4 * qb, 4 * qb + 4):
                osb = work.tile([P, 2, 512], f32, tag="osb")
                for dh in range(2):
                    ps = psp.tile([P, QB], f32, tag="proj")
                    for ci, c in enumerate(corder):
                        nc.tensor.matmul(
                            ps[:, 0:512],
                            lhsT=OTC_t[qt // 4][:, c,
                                                (qt % 4) * P:(qt % 4 + 1) * P],
                            rhs=wo_sb[:, c, dh * 512:(dh + 1) * 512],
                            start=(ci == 0), stop=(ci == 1),
                        )
                    # at the kernel tail ACT is idle: alternate evacuations
                    # across DVE/ACT so the last copies run in parallel
                    if alt_evac and dh == 1:
                        nc.scalar.copy(osb[:, dh, :], ps[:, 0:512])
                    else:
                        nc.vector.tensor_copy(osb[:, dh, :], ps[:, 0:512])
                nc.sync.dma_start(
                    out=out[qt * P:(qt + 1) * P, :], in_=osb)

        # offset-by-one interleave: attention for qb emitted after stage-A
        # block qb+1, so projections keep a one-block head start on the PE
        if mode == "causal":
            # attn(qb) only reads KT/V s-blocks <= qb, all emitted beforehand;
            # wo(qb) interleaved as soon as OTC[qb] is complete so the PE has
            # independent work while late attention chains drain
            emit_stage_a_sb(0)
            emit_attn(0, heads=[1, 0])
            emit_stage_a_sb(1)
            emit_attn(0, heads=[3, 2])
            emit_attn(1, heads=[1, 0])
            emit_stage_a_sb(2)
            emit_attn(1, heads=[3, 2])
            emit_attn(2, heads=[1, 0])
            emit_stage_a_sb(3)
            emit_attn(2, heads=[3, 2])
            emit_wo(0)
            emit_attn(3, heads=[3, 2])
            emit_wo(1)
            emit_attn(3, heads=[1, 0], tail=True)
            emit_wo(2)
            emit_wo(3, corder=(1, 0), alt_evac=True)
        else:
            # non-causal attn reads ALL KT/V tiles: emitting it early would
            # precede their writers (Tile records deps at emission time)
            for _sb in range(NQB):
                emit_stage_a_sb(_sb)
            for _qb in range(NQB):
                emit_attn(_qb)
            for _qb in range(NQB):
                emit_wo(_qb)
    # split multi-wait conditions: TRN2 instructions hold at most one sync
    # wait (EventSemaphore holds two); walrus refuses to split them itself
    import bass_rust
    bass_rust.move_matmul_waits_to_ldweights(nc.m)
    bass_rust.generate_event_semaphores(nc)
    return nc


_NC_CACHE = {}


def kernel(_trace=False, **inputs):
    global LAST_EXEC_NS, LAST_PROFILE
    x = np.ascontiguousarray(np.asarray(inputs["x"], dtype=np.float32))
    wq = np.asarray(inputs["wq"], dtype=np.float32)
    wk = np.asarray(inputs["wk"], dtype=np.float32)
    wv = np.asarray(inputs["wv"], dtype=np.float32)
    wo = np.asarray(inputs["wo"], dtype=np.float32)
    fc = np.asarray(inputs["freqs_cos"], dtype=np.float32)
    fs = np.asarray(inputs["freqs_sin"], dtype=np.float32)
    mask = np.asarray(inputs["mask"], dtype=np.float32)

    mode = _classify_mask(mask)
    if mode not in _NC_CACHE:
        _NC_CACHE[mode] = _build_nc(mode)
    nc = _NC_CACHE[mode]
    in_maps = _make_in_maps(x, wq, wk, wv, wo, fc, fs, mask, mode)

    try:
        res = run_bass_kernel_spmd(
            nc, in_maps, core_ids=list(range(8)), trace=_trace)
    except (ModuleNotFoundError, ImportError):
        res = run_bass_kernel_spmd(
            nc, in_maps, core_ids=list(range(8)), trace=False)
    LAST_EXEC_NS = res.exec_time_ns
    LAST_PROFILE = res.profile_json
    full = np.zeros((B, S, D), dtype=np.float32)
    for b in range(B):
        for g in range(KH):
            full[b] += res.results[b * KH + g]["out"]
    return full


def _make_in_maps(x, wq, wk, wv, wo, fc, fs, mask, mode):
    # head_dim permutation: evens then odds (consistent on q & k -> scores invariant)
    perm = np.concatenate([np.arange(0, HD, 2), np.arange(1, HD, 2)])
    wq_p = wq.reshape(D, H, HD)[:, :, perm].reshape(D, H * HD)
    wk_p = wk.reshape(D, KH, HD)[:, :, perm].reshape(D, KH * HD)

    cosT = fc.T.astype(np.float32)                      # [32, S]
    sinT = fs.T.astype(np.float32)
    cos_rep = np.ascontiguousarray(np.tile(cosT, (4, 1)))          # [128, S]
    sin_signed = np.ascontiguousarray(
        np.concatenate([-sinT, sinT, -sinT, sinT], axis=0))        # [128, S]

    # tri[c, i] = 1 where q-offset i >= key-offset c (keep), else 0
    cc = np.arange(P)[:, None]
    ii = np.arange(P)[None, :]
    tri = (ii >= cc).astype(np.float32)

    import ml_dtypes
    b16 = ml_dtypes.bfloat16

    in_maps = []
    for b in range(B):
        xTb = np.ascontiguousarray(x[b].T).astype(b16).reshape(DCH, P, S)
        for g in range(KH):
            wk_g = wk_p[:, g * HD:(g + 1) * HD]
            wk_dup = np.concatenate([wk_g, wk_g], axis=1)       # [D, 128]
            m = {
                "xT": xTb,
                "wq": np.ascontiguousarray(
                    wq_p[:, g * GH * HD:(g + 1) * GH * HD]
                ).astype(b16).reshape(DCH, P, GH * HD),
                "wk": np.ascontiguousarray(wk_dup).astype(b16).reshape(DCH, P, 2 * HD),
                "wv": np.ascontiguousarray(
                    wv[:, g * HD:(g + 1) * HD]).astype(b16).reshape(DCH, P, HD),
                "wo": np.ascontiguousarray(
                    wo[g * GH * HD:(g + 1) * GH * HD]).astype(b16).reshape(2, P, D),
                "cos": cos_rep.astype(b16),
                "sin": sin_signed.astype(b16),
                "tri": tri.astype(b16),
            }
            if mode == "general":
                m["maskT"] = np.ascontiguousarray(
                    mask.reshape(S, S).T).reshape(NKT, P, S)
            in_maps.append(m)
    return in_maps


# revision 60
# speedup vs baseline: 1.0715x; 1.0248x over previous
"""GQA attention (B=2,S=2048,D=1024,H=16,KH=4,HD=64) + RoPE + causal mask on 8 trn2 cores.

Sharding: core = (batch b, kv-group g).  Each core computes its 4 query heads'
attention against its single KV head and a partial output  O_g @ wo_g  [S, D];
the host sums the 4 partials per batch.

Per-core device pipeline (everything transposed so softmax-sum runs on the PE):
  - host passes x[b]^T so QKV projections contract D on partitions
  - head_dim of wq/wk is permuted on host to [evens, odds] so RoPE is two
    32-row blocks; scores are invariant to a consistent q/k head_dim permutation
  - RoPE: DVE evac (f32 PSUM -> bf16 SBUF), 4 partition-swap DMAs, two bf16
    DVE muls with [cos..]/[-sin,+sin..] tiles, add on GpSimd
  - scores computed transposed  S^T[k, q] = K^T(lhsT) x Q^T(rhs), bf16 matmuls
  - causal handling: per 128-key tile only the q >= k_tile_start slice is
    computed/exp'd; the one triangular 128x128 block per tile is zeroed
    AFTER exp by a cheap DVE mul with a 0/1 triangle tile (no mask matmuls)
  - softmax without max-subtraction (scores bounded); exp on ScalarE w/ scale=1/8
  - AV uses V augmented with a ones column: one accumulating matmul per key
    tile yields both O^T[64, q] and the softmax denominator row
  - normalization: DVE reciprocal of denom row, partition-broadcast DMA of
    1/denom, fused multiply on the PSUM->SBUF copy; even heads' multiply
    writes the packed OTC tile rows 0..64 directly (odd heads need one
    partition-move DMA to rows 64..128 - matmul out base must be 0/32/64)
  - wo projection consumes O^T chunks directly as lhsT; PSUM -> SBUF pair ->
    one DRAM store per 128-row block; emission interleaves wo blocks between
    late attention chunks so the PE stays fed through the tail
"""

import os
import sys

import numpy as np

for _p in ("/opt/trn_rl_repo", "/root/.axon_site/_ro/trn_rl_repo"):
    if os.path.isdir(_p) and _p not in sys.path:
        sys.path.insert(0, _p)

from contextlib import ExitStack

import concourse.bass as bass
import concourse.tile as tile
from concourse import bacc as _bacc
from concourse import mybir
from concourse.bass_utils import run_bass_kernel_spmd

B, S, D = 2, 2048, 1024
H, KH, HD = 16, 4, 64
REP = H // KH          # 4 query heads per kv head
GH = REP               # heads per core
P = 128
QB = 512               # q block (matmul moving free dim)
NKT = S // P           # 16 key tiles
NQB = S // QB          # 4 q blocks
DCH = D // P           # 8 contraction chunks for D

f32 = mybir.dt.float32
f32r = mybir.dt.float32r
bf16 = mybir.dt.bfloat16

LAST_EXEC_NS = None
LAST_PROFILE = None


def _classify_mask(mask):
    m = np.asarray(mask).reshape(S, S)
    if not m.any():
        return "none"
    tril = np.tril(np.ones((S, S), dtype=bool))
    if (m[tril] == 0.0).all() and (m[~tril] < -1e30).all():
        return "causal"
    return "general"


def _build_nc(mode):
    import os as _os
    _skip = set(_os.environ.get("KABLATE", "").split(","))
    nc = bass.Bass()
    xT = nc.declare_dram_parameter("xT", [DCH, P, S], bf16, isOutput=False)
    wq = nc.declare_dram_parameter("wq", [DCH, P, GH * HD], bf16, isOutput=False)
    wk = nc.declare_dram_parameter("wk", [DCH, P, 2 * HD], bf16, isOutput=False)
    wv = nc.declare_dram_parameter("wv", [DCH, P, HD], bf16, isOutput=False)
    wo = nc.declare_dram_parameter("wo", [2, P, D], bf16, isOutput=False)
    cos = nc.declare_dram_parameter("cos", [P, S], bf16, isOutput=False)
    sin = nc.declare_dram_parameter("sin", [P, S], bf16, isOutput=False)
    tri = nc.declare_dram_parameter("tri", [P, P], bf16, isOutput=False)
    if mode == "general":
        maskT = nc.declare_dram_parameter("maskT", [NKT, P, S], f32, isOutput=False)
    out = nc.declare_dram_parameter("out", [S, D], f32, isOutput=True)

    with tile.TileContext(nc) as tc, ExitStack() as ctx:
        const = ctx.enter_context(tc.tile_pool(name="const", bufs=1))
        big = ctx.enter_context(tc.tile_pool(name="big", bufs=1))
        work = ctx.enter_context(tc.tile_pool(name="work", bufs=6))
        ptp = ctx.enter_context(tc.tile_pool(name="ptp", bufs=8))
        psp = ctx.enter_context(tc.tile_pool(name="psp", bufs=2, space="PSUM"))
        stp = ctx.enter_context(tc.tile_pool(name="stp", bufs=2, space="PSUM"))
        avp = ctx.enter_context(tc.tile_pool(name="avp", bufs=2, space="PSUM"))

        # ---- constants / weights to SBUF ----
        # few DMAs per tile: consumers wait per DMA-queue semaphore, and walrus
        # rejects instructions with too many wait conditions
        # preload order matters: the first Q projection + rope + first scores
        # need wq/xt[sb0]/wk/wv/tri/cos/sin - queue those first so the PE
        # doesn't idle behind the bulk of the xt/wo loads
        xt_sb = big.tile([P, DCH, S], bf16, tag="xt")
        wq_sb = const.tile([P, DCH, GH * HD], bf16, tag="wq")
        wk_sb = const.tile([P, DCH, 2 * HD], bf16, tag="wk")
        wv_sb = const.tile([P, DCH, HD], bf16, tag="wv")
        wo_sb = const.tile([P, 2, D], bf16, tag="wo")
        cos_sb = const.tile([P, S], bf16, tag="cos")
        sin_sb = const.tile([P, S], bf16, tag="sin")
        tri_sb = const.tile([P, P], bf16, tag="tri")

        def _xt_load(sb, half=None):
            sl = slice(sb * QB, (sb + 1) * QB)
            ch = slice(0, DCH) if half is None else (
                slice(0, DCH // 2) if half == 0 else slice(DCH // 2, DCH))
            nc.sync.dma_start(
                out=xt_sb[:, ch, sl],
                in_=xT[ch, :, sl].rearrange("c p s -> p c s"),
            )
        nc.sync.dma_start(out=wk_sb, in_=wk[:, :, :].rearrange("c p f -> p c f"))
        _xt_load(0, half=0)
        _xt_load(0, half=1)
        nc.sync.dma_start(out=wq_sb, in_=wq[:, :, :].rearrange("c p f -> p c f"))
        nc.sync.dma_start(out=sin_sb, in_=sin[:, :])
        nc.sync.dma_start(out=cos_sb, in_=cos[:, :])
        nc.sync.dma_start(out=wv_sb, in_=wv[:, :, :].rearrange("c p f -> p c f"))
        nc.sync.dma_start(out=tri_sb, in_=tri[:, :])
        for _sb in range(1, NQB):
            _xt_load(_sb)
        nc.sync.dma_start(out=wo_sb, in_=wo[:, :, :].rearrange("c p f -> p c f"))

        # per-s-block tiles: fine-grained deps let attention start as soon as
        # the first s-block of Q/K/V is ready instead of after all of stage A
        QT_t = [big.tile([P, 2, QB], bf16, tag=f"QT{i}", name=f"QT{i}") for i in range(NQB)]
        KT_t = [big.tile([P, QB], bf16, tag=f"KT{i}", name=f"KT{i}") for i in range(NQB)]
        # V augmented with a ones column: cols [V(64), ones]
        V_t = [big.tile([P, 4, HD + 1], bf16, tag=f"V{i}", name=f"V{i}") for i in range(NQB)]
        OTC_t = [big.tile([P, 2, QB], bf16, tag=f"OTC{i}", name=f"OTC{i}") for i in range(NQB)]
        for i in range(NQB):
            nc.vector.memset(V_t[i][:, :, HD:HD + 1], 1.0)

        def rope(ps, out_ap, sl):
            # ps rows: per 64-group [evens(32), odds(32)]; swap 32-row halves.
            # DMA cannot read PSUM, so evacuate via DVE copy (cast to bf16).
            sb_ps = work.tile([P, QB], bf16, tag="ropesb")
            nc.vector.tensor_copy(sb_ps, ps)
            tmp = work.tile([P, QB], bf16, tag="ropetmp")
            for r0 in range(0, P, 64):
                nc.sync.dma_start(
                    out=tmp[r0:r0 + 32, :], in_=sb_ps[r0 + 32:r0 + 64, :])
                nc.sync.dma_start(
                    out=tmp[r0 + 32:r0 + 64, :], in_=sb_ps[r0:r0 + 32, :])
            ta = work.tile([P, QB], bf16, tag="ropeta")
            tb = work.tile([P, QB], bf16, tag="ropetb")
            nc.vector.tensor_mul(ta, sb_ps, cos_sb[:, sl])
            nc.vector.tensor_mul(tb, tmp, sin_sb[:, sl])
            nc.gpsimd.tensor_add(out_ap, ta, tb)

        # ---- Q/K/V per s-block (emission order lets qb0 attention start early)
        def emit_stage_a_sb(sb):
          if True:
              sl = slice(sb * QB, (sb + 1) * QB)
              # K first: wk is half the size of wq, so the K->rope->KT chain
              # that gates the first scores starts earliest
              ps = psp.tile([P, QB], f32, tag="proj")
              for dc in range(DCH):
                  nc.tensor.matmul(
                      ps, lhsT=wk_sb[:, dc, :], rhs=xt_sb[:, dc, sl],
                      start=(dc == 0), stop=(dc == DCH - 1),
                  )
              rope(ps, KT_t[sb], sl)
              for ch in range(2):
                  ps = psp.tile([P, QB], f32, tag="proj")
                  for dc in range(DCH):
                      nc.tensor.matmul(
                          ps, lhsT=wq_sb[:, dc, ch * P:(ch + 1) * P],
                          rhs=xt_sb[:, dc, sl],
                          start=(dc == 0), stop=(dc == DCH - 1),
                      )
                  rope(ps, QT_t[sb][:, ch, :], sl)
              for st_i in range(4 * sb, 4 * sb + 4):
                  ps = psp.tile([P, HD], f32, tag="proj")
                  for dc in range(DCH):
                      nc.tensor.matmul(
                          ps, lhsT=xt_sb[:, dc, st_i * P:(st_i + 1) * P],
                          rhs=wv_sb[:, dc, :],
                          start=(dc == 0), stop=(dc == DCH - 1),
                      )
                  nc.vector.tensor_copy(V_t[sb][:, st_i - 4 * sb, 0:HD], ps)

        # ---- attention per (head, q block), emitted in wavefront order ----
        def emit_attn(qb, heads=None, tail=False):
            # odd head first within each chunk: the chunk's last OTC write is
            # then the even head's direct DVE write (no partition-move DMA)
            for h in (heads if heads is not None
                      else ([1, 0, 3, 2] if "B" not in _skip else [])):
                ch, hr = h // 2, (h % 2) * 64
                q0 = qb * QB
                qsl = slice(q0, q0 + QB)
                nk = 4 * (qb + 1) if mode == "causal" else NKT
                # AV rows: [O(0..64), denom@64] <- V cols [V, ones]; for even
                # heads (hr=0) the normalize mul then writes OTC rows 0..64
                # directly; odd heads need a partition-move DMA to rows 64..128
                # (matmul out base partition must be 0/32/64, so the odd-head
                # result cannot be placed at its OTC rows directly)
                av = avp.tile([P, QB], f32, tag="av")
                av_out = av[0:HD + 1, :]
                denom, orows = av[HD:HD + 1, :], av[0:HD, :]
                for kt0 in range(0, nk, 2):
                    st = stp.tile([P, 2, QB], f32, tag="st")
                    offs = []
                    for j in range(2):
                        kt = kt0 + j
                        # causal: keys [128kt, 128kt+128) only see q >= 128kt
                        # (within this q block) -> slice the moving dim
                        diag = mode == "causal" and kt >= nk - 4
                        o = (kt - (nk - 4)) * P if diag else 0
                        offs.append(o)
                        nc.tensor.matmul(
                            st[:, j, o:],
                            lhsT=KT_t[kt // 4][hr:hr + 64,
                                               (kt % 4) * P:(kt % 4 + 1) * P],
                            rhs=QT_t[qb][hr:hr + 64, ch, o:],
                            start=True, stop=True,
                        )
                        if mode == "general":
                            mt = work.tile([P, QB], f32, tag="maskt")
                            nc.sync.dma_start(out=mt, in_=maskT[kt, :, qsl])
                            nc.vector.tensor_add(st[:, j, :], st[:, j, :], mt)
                    if "E" in _skip:
                        continue
                    pt = ptp.tile([P, 2, QB], bf16, tag="pt")
                    if offs[0] == offs[1]:
                        nc.scalar.activation(
                            pt[:, :, offs[0]:], st[:, :, offs[0]:],
                            mybir.ActivationFunctionType.Exp, scale=0.125,
                        )
                    else:
                        for j in range(2):
                            nc.scalar.activation(
                                pt[:, j, offs[j]:], st[:, j, offs[j]:],
                                mybir.ActivationFunctionType.Exp, scale=0.125,
                            )
                    for j in range(2):
                        kt = kt0 + j
                        o = offs[j]
                        if mode == "causal" and kt >= nk - 4:
                            # zero the above-diagonal part of the one
                            # triangular 128x128 block (post-exp mask)
                            nc.vector.tensor_mul(
                                pt[:, j, o:o + P], pt[:, j, o:o + P], tri_sb)
                        nc.tensor.matmul(
                            av_out[:, o:],
                            lhsT=V_t[kt // 4][:, kt % 4, :],
                            rhs=pt[:, j, o:],
                            start=(kt == 0), stop=(kt == nk - 1),
                        )
                # normalize: r = 1/denom; replicate across 64 partitions with a
                # partition-step-0 DMA source AP; the fused multiply on the
                # PSUM->SBUF copy writes the packed OTC tile directly
                r1 = work.tile([1, QB], f32, tag="r1")
                rbs = work.tile([64, QB], f32, tag="rbs")
                ot = (None if hr == 0 else
                      work.tile([64, QB], bf16, tag="ot"))
                # tail heads split the normalize into column halves: denom
                # cols < 128*(o+1) are final before the last AV matmuls, so
                # the first half's recip->bcast->mul chain hides under them
                # and half the wo stop-matmuls unblock earlier
                for hsl in ([slice(0, QB // 2), slice(QB // 2, QB)]
                            if tail else [slice(0, QB)]):
                    nc.vector.reciprocal(r1[:, hsl], denom[:, hsl])
                    # replicate 1/denom across 64 partitions with a
                    # partition-step-0 DMA source AP (SWDGE: own queue, keeps
                    # the SP/HWDGE stream free; a DVE op may read only one
                    # non-scalar PSUM operand -> broadcast must land in SBUF)
                    r1h = r1[:, hsl]
                    r1b = bass.AP(tensor=r1h.tensor, offset=r1h.offset,
                                  ap=[list(r1h.ap[0]), [0, 64]]
                                  + [list(a) for a in r1h.ap[1:]])
                    with nc.allow_non_contiguous_dma(
                            reason="partition broadcast"):
                        # tail chains use HWDGE (lower latency; SP idle there)
                        (nc.sync if tail else nc.gpsimd).dma_start(
                            out=rbs[:, hsl], in_=r1b)
                    if hr == 0:
                        nc.vector.tensor_mul(
                            OTC_t[qb][0:64, ch, hsl], orows[:, hsl],
                            rbs[:, hsl])
                    else:
                        nc.vector.tensor_mul(ot[:, hsl], orows[:, hsl],
                                             rbs[:, hsl])
                if hr != 0:
                    # gpsimd SWDGE is pinned to one queue -> single wait
                    # condition for the wo matmuls that consume OTC
                    nc.gpsimd.dma_start(out=OTC_t[qb][64:128, ch, :], in_=ot)

        # ---- output projection: out[q,:] = sum_c OTC[:,c,q].T @ wo[c] ----
        def emit_wo(qb, corder=(0, 1), alt_evac=False):
            if "C" in _skip:
                return
            for qt in range(4 * qb, 4 * qb + 4):
                osb = work.tile([P, 2, 512], f32, tag="osb")
                for dh in range(2):
                    ps = psp.tile([P, QB], f32, tag="proj")
                    for ci, c in enumerate(corder):
                        nc.tensor.matmul(
                            ps[:, 0:512],
                            lhsT=OTC_t[qt // 4][:, c,
                                                (qt % 4) * P:(qt % 4 + 1) * P],
                            rhs=wo_sb[:, c, dh * 512:(dh + 1) * 512],
                            start=(ci == 0), stop=(ci == 1),
                        )
                    # at the kernel tail ACT is idle: alternate evacuations
                    # across DVE/ACT so the last copies run in parallel
                    if alt_evac and dh == 1:
                        nc.scalar.copy(osb[:, dh, :], ps[:, 0:512])
                    else:
                        nc.vector.tensor_copy(osb[:, dh, :], ps[:, 0:512])
                nc.sync.dma_start(
                    out=out[qt * P:(qt + 1) * P, :], in_=osb)

        # offset-by-one interleave: attention for qb emitted after stage-A
        # block qb+1, so projections keep a one-block head start on the PE
        if mode == "causal":
            # attn(qb) only reads KT/V s-blocks <= qb, all emitted beforehand;
            # wo(qb) interleaved as soon as OTC[qb] is complete so the PE has
            # independent work while late attention chains drain
            emit_stage_a_sb(0)
            emit_attn(0, heads=[1, 0])
            emit_stage_a_sb(1)
            emit_attn(0, heads=[3, 2])
            emit_attn(1, heads=[1, 0])
            emit_stage_a_sb(2)
            emit_attn(1, heads=[3, 2])
            emit_attn(2, heads=[1, 0])
            emit_stage_a_sb(3)
            emit_attn(2, heads=[3, 2])
            emit_wo(0)
            emit_attn(3, heads=[3, 2])
            emit_wo(1)
            emit_attn(3, heads=[1, 0], tail=True)
            emit_wo(2)
            emit_wo(3, corder=(1, 0), alt_evac=True)
        else:
            # non-causal attn reads ALL KT/V tiles: emitting it early would
            # precede their writers (Tile records deps at emission time)
            for _sb in range(NQB):
                emit_stage_a_sb(_sb)
            for _qb in range(NQB):
                emit_attn(_qb)
            for _qb in range(NQB):
                emit_wo(_qb)
    # split multi-wait conditions: TRN2 instructions hold at most one sync
    # wait (EventSemaphore holds two); walrus refuses to split them itself
    import bass_rust
    bass_rust.move_matmul_waits_to_ldweights(nc.m)
    bass_rust.generate_event_semaphores(nc)
    return nc


_NC_CACHE = {}


def kernel(_trace=False, **inputs):
    global LAST_EXEC_NS, LAST_PROFILE
    x = np.ascontiguousarray(np.asarray(inputs["x"], dtype=np.float32))
    wq = np.asarray(inputs["wq"], dtype=np.float32)
    wk = np.asarray(inputs["wk"], dtype=np.float32)
    wv = np.asarray(inputs["wv"], dtype=np.float32)
    wo = np.asarray(inputs["wo"], dtype=np.float32)
    fc = np.asarray(inputs["freqs_cos"], dtype=np.float32)
    fs = np.asarray(inputs["freqs_sin"], dtype=np.float32)
    mask = np.asarray(inputs["mask"], dtype=np.float32)

    mode = _classify_mask(mask)
    if mode not in _NC_CACHE:
        _NC_CACHE[mode] = _build_nc(mode)
    nc = _NC_CACHE[mode]
    in_maps = _make_in_maps(x, wq, wk, wv, wo, fc, fs, mask, mode)

    try:
        res = run_bass_kernel_spmd(
            nc, in_maps, core_ids=list(range(8)), trace=_trace)
    except (ModuleNotFoundError, ImportError):
        res = run_bass_kernel_spmd(
            nc, in_maps, core_ids=list(range(8)), trace=False)
    LAST_EXEC_NS = res.exec_time_ns
    LAST_PROFILE = res.profile_json
    full = np.zeros((B, S, D), dtype=np.float32)
    for b in range(B):
        for g in range(KH):
            full[b] += res.results[b * KH + g]["out"]
    return full


def _make_in_maps(x, wq, wk, wv, wo, fc, fs, mask, mode):
    # head_dim permutation: evens then odds (consistent on q & k -> scores invariant)
    perm = np.concatenate([np.arange(0, HD, 2), np.arange(1, HD, 2)])
    wq_p = wq.reshape(D, H, HD)[:, :, perm].reshape(D, H * HD)
    wk_p = wk.reshape(D, KH, HD)[:, :, perm].reshape(D, KH * HD)

    cosT = fc.T.astype(np.float32)                      # [32, S]
    sinT = fs.T.astype(np.float32)
    cos_rep = np.ascontiguousarray(np.tile(cosT, (4, 1)))          # [128, S]
    sin_signed = np.ascontiguousarray(
        np.concatenate([-sinT, sinT, -sinT, sinT], axis=0))        # [128, S]

    # tri[c, i] = 1 where q-offset i >= key-offset c (keep), else 0
    cc = np.arange(P)[:, None]
    ii = np.arange(P)[None, :]
    tri = (ii >= cc).astype(np.float32)

    import ml_dtypes
    b16 = ml_dtypes.bfloat16

    in_maps = []
    for b in range(B):
        xTb = np.ascontiguousarray(x[b].T).astype(b16).reshape(DCH, P, S)
        for g in range(KH):
            wk_g = wk_p[:, g * HD:(g + 1) * HD]
            wk_dup = np.concatenate([wk_g, wk_g], axis=1)       # [D, 128]
            m = {
                "xT": xTb,
                "wq": np.ascontiguousarray(
                    wq_p[:, g * GH * HD:(g + 1) * GH * HD]
                ).astype(b16).reshape(DCH, P, GH * HD),
                "wk": np.ascontiguousarray(wk_dup).astype(b16).reshape(DCH, P, 2 * HD),
                "wv": np.ascontiguousarray(
                    wv[:, g * HD:(g + 1) * HD]).astype(b16).reshape(DCH, P, HD),
                "wo": np.ascontiguousarray(
                    wo[g * GH * HD:(g + 1) * GH * HD]).astype(b16).reshape(2, P, D),
                "cos": cos_rep.astype(b16),
                "sin": sin_signed.astype(b16),
                "tri": tri.astype(b16),
            }
            if mode == "general":
                m["maskT"] = np.ascontiguousarray(
                    mask.reshape(S, S).T).reshape(NKT, P, S)
            in_maps.append(m)
    return in_maps
